# revision 12
# baseline (speedup 1.0000x reference)
"""HOSVD aggregator kernel for 8 TRN2 NeuronCores.

y[n,o] = sum_{m0..m4} G[m0,m1,m2,m3,m4] * ris0[n,m0] * ris1[n,m4]
         * ris2[n,m3] * ris3[n,m2] * U_out[m1,o],
with ris_d = X[:,d,:] @ U_stack[d].

Data-parallel over nodes (6250/core), transposed layout (features on
partitions, nodes on the free dim). Per 512-node supertile:
  - 8 factor matmuls (K=128 each) fill two PSUM banks:
      psAA = [A01-64 | A23-64], psBB = [B01-64 | B23-64]
    via host-interleaved input chunks (d0/d2 and d1/d3 64-feature blocks
    stacked on partitions) and expanded factor weights.
  - one ScalarE copy (psAA -> bf16) + one VectorE mul gives
      z = [z01 | z23] in one op.
  - GpSimd builds the partition-duplicated forms (z01 on rows 64-127,
    z23 tiled on all 128 rows).
  - PT = G2 @ z01 runs as 2 concurrent row-tiled matmul pairs (K=64,
    tile_position (0,0)/(64,0)) -> 4 chunks in 2 PE slots.
  - Q = PT * z23t on VectorE (2 chunks direct from PSUM, 2 staged to
    bf16 SBUF by ScalarE for the 2x DVE mode), then 4 accumulating
    y-matmuls with repeat-expanded U_out.
  - Output stored bf16 (host upcasts to f32).
14 PE slots/supertile; Vector/Scalar ~2.7us each; PE-bound steady state.
"""

import sys

sys.path.insert(0, "/opt/trn_rl_repo")

import os
import numpy as np
import ml_dtypes

import concourse.bass as bass
import concourse.tile as tile
from concourse import mybir
from concourse.bass_utils import run_bass_kernel_spmd

BF16 = ml_dtypes.bfloat16

N = 50000
NCORES = 8
NPC = N // NCORES            # 6250 nodes per core
T = 512                      # nodes per supertile
NSUP = NPC // T              # 12 full supertiles
TAIL = NPC - NSUP * T        # 106

# ---------------------------------------------------------------------------
# walrus rejects >1 sync wait on a Drain; Tile's tail drain carries one wait
# per logical proc. Split it into a chain of single-wait drains.
import bass_rust as _br
from concourse.vector_clock import ScopedClock as _ScopedClock


def _split_drain_and_barrier(self, tick_clock, wait_clock):
    drain_inst = self.nc.sync.drain()
    wait_clock.add_sem_waits(
        drain_inst.ins, _ScopedClock({None: tick_clock.global_clock})
    )
    si = drain_inst.ins.sync_info
    waits = list(si.on_wait)
    if len(waits) > 1:
        drain_inst.ins.sync_info = _br.SyncInfo(on_wait=waits[:1], on_update=[])
        rest = waits[1:]
        while rest:
            d2 = self.nc.sync.drain()
            chunk, rest = rest[:1], rest[1:]
            d2.ins.sync_info = _br.SyncInfo(
                on_wait=chunk, on_update=list(si.on_update) if not rest else []
            )
    self.nc.all_engine_barrier()
    assert self.sems is not None
    popped = self.nc._tile_sem_poison_stack.pop()
    assert popped is self._sem_poison
    self.nc.clear_and_free_semaphores(list(self.sems.allocated().values()))
    self.nc.all_engine_barrier()


tile.TileContext._drain_and_barrier = _split_drain_and_barrier

# Same walrus limit applies to every instruction type: peel extra sem waits
# onto single-wait NOPs emitted just before the instruction, same engine.
_SPLIT_SEQ = [0]
_orig_add_instruction = tile.TileContext._add_instruction


def _split_add_instruction(self, inst):
    si = inst.sync_info
    waits = list(si.on_wait) if si is not None else []
    if len(waits) > 1:
        for w in waits[:-1]:
            _SPLIT_SEQ[0] += 1
            nop = mybir.InstNoOp(name=f"waitsplit_{_SPLIT_SEQ[0]}", ins=[],
                                 outs=[], engine=inst.engine)
            nop.sync_info = _br.SyncInfo(on_wait=[w], on_update=[])
            _orig_add_instruction(self, nop)
        inst.sync_info = _br.SyncInfo(on_wait=[waits[-1]],
                                      on_update=list(si.on_update))
    return _orig_add_instruction(self, inst)


tile.TileContext._add_instruction = _split_add_instruction

# ---------------------------------------------------------------------------
# weight-pack free-dim offsets (all bf16, one [128, 1792] SBUF tile)
_WAA = 0      # 4 chunks [128,128]: rows 0-63 U0 block (A01), 64-127 U2 (A23)
_WBB = 512    # 4 chunks [128,128]: rows 0-63 U1 block (B01), 64-127 U3 (B23)
_G2 = 1024    # 2 chunks [128,128]: pair p: rows 0-63 G2 chunk 2p, 64-127 chunk 2p+1
_UE = 1280    # 4 chunks [128,128]: U_out repeat-expanded over (m3,m2)
_WCOLS = 1792


def _build_nc():
    nc = bass.Bass("TRN2", target_bir_lowering=False, debug=False,
                   num_devices=NCORES)
    bf = mybir.dt.bfloat16
    f32 = mybir.dt.float32

    xm = nc.dram_tensor("xm", [NSUP, 128, 8 * T], bf, kind="ExternalInput").ap()
    xt = nc.dram_tensor("xt", [128, 8 * TAIL], bf, kind="ExternalInput").ap()
    wp = nc.dram_tensor("wp", [128, _WCOLS], bf, kind="ExternalInput").ap()
    ym = nc.dram_tensor("ym", [NSUP, 128, T], bf, kind="ExternalOutput").ap()
    yt = nc.dram_tensor("yt", [128, TAIL], bf, kind="ExternalOutput").ap()

    NT = NSUP + 1

    with tile.TileContext(nc) as tc:
        from contextlib import ExitStack
        with ExitStack() as ctx:
            wpool = ctx.enter_context(tc.tile_pool(name="w", bufs=1))
            xpool = ctx.enter_context(tc.tile_pool(name="x", bufs=5))
            spool = ctx.enter_context(tc.tile_pool(name="s", bufs=2))
            zpool = ctx.enter_context(tc.tile_pool(name="z", bufs=2))
            hpool = ctx.enter_context(tc.tile_pool(name="h", bufs=2))
            tpool = ctx.enter_context(tc.tile_pool(name="t", bufs=3))
            qpool = ctx.enter_context(tc.tile_pool(name="q", bufs=2))
            ypool = ctx.enter_context(tc.tile_pool(name="y", bufs=2))
            # PSUM: psAA 1 + psBB 2 + ptqA 2 + ptqB 2 + psy 1 = 8 banks
            fpsA = ctx.enter_context(tc.tile_pool(name="pa", bufs=1, space="PSUM"))
            fpsB = ctx.enter_context(tc.tile_pool(name="pb", bufs=2, space="PSUM"))
            ppsum = ctx.enter_context(tc.tile_pool(name="pp", bufs=2, space="PSUM"))
            ypsum = ctx.enter_context(tc.tile_pool(name="py", bufs=1, space="PSUM"))

            ws = wpool.tile([128, _WCOLS], bf)

            # PE pre-warm: ~4.3us of garbage matmuls while the first DMAs are
            # in flight flips HAM to K=8/8 (needs >3.4us sustained busy)
            # before the real matmuls arrive
            dum = wpool.tile([128, 128], bf)
            nc.gpsimd.memset(dum[:], 0)
            dps = ypsum.tile([128, 512], f32, tag="psy")
            for _ in range(40):
                nc.tensor.matmul(dps[:, 0:128], dum[:], dum[:],
                                 start=True, stop=True)

            # per-tile live objects, keyed by tile index
            XS, PSAA, PSBB, SAA, Z, ZXH, Z23T = {}, {}, {}, {}, {}, {}, {}
            PTQ, SQ, QT, PSY = {}, {}, {}, {}

            def tcnt(t):
                return T if t < NSUP else TAIL

            # software-pipelined emission: stage skew gives every cross-engine
            # dependency a full iteration of slack
            for i in range(NT + 4):
                # --- stage A (tile i): input DMA + factor matmuls [PE] ---
                if i < NT:
                    tc_ = tcnt(i)
                    xs = xpool.tile([128, 8 * tc_], bf, tag="xs")
                    XS[i] = xs
                    src = xm[i] if i < NSUP else xt[:]
                    if i == 0:
                        def xq(h):
                            nc.sync.dma_start(xs[:, 2 * tc_ * h:2 * tc_ * (h + 1)],
                                              src[:, 2 * tc_ * h:2 * tc_ * (h + 1)])
                        nc.sync.dma_start(ws[:, _WAA:_WBB], wp[:, _WAA:_WBB])
                        xq(0)
                        nc.sync.dma_start(ws[:, _WBB:_G2], wp[:, _WBB:_G2])
                        xq(1)
                        nc.sync.dma_start(ws[:, _G2:_WCOLS], wp[:, _G2:_WCOLS])
                        xq(2)
                        xq(3)
                    else:
                        nc.sync.dma_start(xs[:], src)
                    psAA = fpsA.tile([128, tc_], f32, tag="psAA")
                    PSAA[i] = psAA
                    psBB = fpsB.tile([128, tc_], f32, tag="psBB")
                    PSBB[i] = psBB
                    for j in range(4):
                        nc.tensor.matmul(psAA[:], ws[:, _WAA + 128 * j:_WAA + 128 * (j + 1)],
                                         xs[:, 2 * j * tc_:(2 * j + 1) * tc_],
                                         start=(j == 0), stop=(j == 3))
                    for j in range(4):
                        nc.tensor.matmul(psBB[:], ws[:, _WBB + 128 * j:_WBB + 128 * (j + 1)],
                                         xs[:, (2 * j + 1) * tc_:(2 * j + 2) * tc_],
                                         start=(j == 0), stop=(j == 3))

                # --- stage E (tile i-4): y accumulation [PE] ---
                t = i - 4
                if 0 <= t < NT:
                    tc_ = tcnt(t)
                    psy = ypsum.tile([128, tc_], f32, tag="psy")
                    PSY[t] = psy
                    for c in range(4):
                        nc.tensor.matmul(psy[:], ws[:, _UE + 128 * c:_UE + 128 * (c + 1)],
                                         QT[t][c][:], start=(c == 0), stop=(c == 3))

                # --- stage C (tile i-2): PT pairs [PE] ---
                t = i - 2
                if 0 <= t < NT:
                    tc_ = tcnt(t)
                    PTQ[t] = {}
                    for p in range(2):
                        ptqA = ppsum.tile([128, tc_], f32, tag="ptqA")
                        PTQ[t][2 * p] = ptqA
                        ptqB = ppsum.tile([128, tc_], f32, tag="ptqB")
                        PTQ[t][2 * p + 1] = ptqB
                        nc.tensor.matmul(ptqA[:], ws[0:64, _G2 + 128 * p:_G2 + 128 * (p + 1)],
                                         Z[t][0:64, :], start=True, stop=True,
                                         tile_position=(0, 0))
                        nc.tensor.matmul(ptqB[:], ws[64:128, _G2 + 128 * p:_G2 + 128 * (p + 1)],
                                         ZXH[t][64:128, :], start=True, stop=True,
                                         tile_position=(64, 0))

                # --- ScalarE queue: sq2/3(i-3), sAA(i), ys(i-4) ---
                t = i - 3
                if 0 <= t < NT:
                    tc_ = tcnt(t)
                    SQ[t] = {}
                    for c in (2, 3):
                        sq = qpool.tile([128, tc_], bf, tag=f"sq{c}")
                        SQ[t][c] = sq
                        nc.scalar.copy(sq[:], PTQ[t][c][:])
                if i < NT:
                    tc_ = tcnt(i)
                    sAA = spool.tile([128, tc_], bf, tag="sAA")
                    SAA[i] = sAA
                    nc.scalar.copy(sAA[:], PSAA[i][:])
                t = i - 4
                if 0 <= t < NT:
                    tc_ = tcnt(t)
                    ys = ypool.tile([128, tc_], bf, tag="ys")
                    nc.scalar.copy(ys[:], PSY[t][:])
                    nc.sync.dma_start(ym[t] if t < NSUP else yt[:], ys[:])

                # --- VectorE queue: z(i-1), z23t dups(i-1), qt0-3(i-3) ---
                t = i - 1
                if 0 <= t < NT:
                    tc_ = tcnt(t)
                    z = zpool.tile([128, tc_], bf, tag="z")
                    Z[t] = z
                    nc.vector.tensor_mul(z[:], PSBB[t][:], SAA[t][:])
                    z23t = tpool.tile([128, tc_], bf, tag="z23t")
                    Z23T[t] = z23t
                    nc.vector.tensor_copy(z23t[0:64, :], z[64:128, :])
                    nc.vector.tensor_copy(z23t[64:128, :], z[64:128, :])
                    # z01 duplicated onto rows 64-127. NOT GpSimd: its copies
                    # are ~1.8us AND the shared SBUF port pair blocks DVE's
                    # 4x-mode copies entirely while GpSimd runs (measured).
                    zxh = hpool.tile([128, tc_], bf, tag="zxh")
                    ZXH[t] = zxh
                    nc.scalar.copy(zxh[64:128, :], z[0:64, :])
                t = i - 3
                if 0 <= t < NT:
                    tc_ = tcnt(t)
                    QT[t] = {}
                    for c in range(4):
                        qt = qpool.tile([128, tc_], bf, tag=f"qt{c}")
                        QT[t][c] = qt
                        if c < 2:
                            nc.vector.tensor_mul(qt[:], PTQ[t][c][:], Z23T[t][:])
                        else:
                            nc.vector.tensor_mul(qt[:], SQ[t][c][:], Z23T[t][:])
    return nc


def _host_pack_weights(G, U_stack, U_output):
    U = np.asarray(U_stack, np.float32)
    Uo = np.asarray(U_output, np.float32)
    Gf = np.asarray(G, np.float32)
    wpk = np.zeros((128, _WCOLS), np.float32)
    for j in range(4):
        fb = slice(64 * j, 64 * j + 64)
        wpk[0:64, _WAA + 128 * j:_WAA + 128 * j + 64] = np.repeat(U[0][fb], 8, axis=1)
        wpk[64:128, _WAA + 128 * j + 64:_WAA + 128 * (j + 1)] = np.repeat(U[2][fb], 8, axis=1)
        wpk[0:64, _WBB + 128 * j:_WBB + 128 * j + 64] = np.tile(U[1][fb], (1, 8))
        wpk[64:128, _WBB + 128 * j + 64:_WBB + 128 * (j + 1)] = np.tile(U[3][fb], (1, 8))
    G2 = np.ascontiguousarray(Gf.transpose(0, 4, 1, 3, 2)).reshape(64, 512)
    for p in range(2):
        wpk[0:64, _G2 + 128 * p:_G2 + 128 * (p + 1)] = G2[:, 256 * p:256 * p + 128]
        wpk[64:128, _G2 + 128 * p:_G2 + 128 * (p + 1)] = G2[:, 256 * p + 128:256 * p + 256]
    Uexp = np.repeat(Uo, 64, axis=0)           # [512,128]
    for c in range(4):
        wpk[:, _UE + 128 * c:_UE + 128 * (c + 1)] = Uexp[128 * c:128 * (c + 1)]
    return wpk.astype(BF16)


def _pack_x_core(sh):
    """sh [NPC, 4, 256] f32 -> (xm [NSUP,128,8T], xt [128,8*TAIL]) bf16.
    Column chunk c=2j+g: g=0 AA (d0 feats p<64, d2 p>=64), g=1 BB (d1, d3);
    within chunk j, partition p -> feature 64*j + p%64."""
    def pack(block, tcnt):
        b = block.reshape(tcnt, 4, 4, 64)             # [t, d, j, f]
        a = b[:, [0, 2], :, :].transpose(1, 3, 2, 0).reshape(128, 4, tcnt)
        bb = b[:, [1, 3], :, :].transpose(1, 3, 2, 0).reshape(128, 4, tcnt)
        xs = np.stack([a, bb], axis=2)                # [128, j, g, t]
        return np.ascontiguousarray(xs.reshape(128, 8 * tcnt))
    main = np.stack([pack(sh[s * T:(s + 1) * T], T) for s in range(NSUP)])
    tail = pack(sh[NSUP * T:], TAIL)
    return main.astype(BF16), tail.astype(BF16)


def _install_ntff_hook():
    import types
    if "antenv.axon_hooks" in sys.modules:
        return
    mod = types.ModuleType("antenv.axon_hooks")
    holder = {"hook": None}
    mod.set_axon_ntff_profile_hook = lambda h: holder.__setitem__("hook", h)
    mod.get_axon_ntff_profile_hook = lambda: holder["hook"]
    sys.modules["antenv.axon_hooks"] = mod
    import antenv
    antenv.axon_hooks = mod
    from trn_agent_boot.trn_boot import _ntff_profile_via_ctypes
    mod.set_axon_ntff_profile_hook(_ntff_profile_via_ctypes("/opt/axon/libaxon_pjrt.so"))


_NC_CACHE = None


def kernel(neighbour_states, G, U_stack, U_output):
    global _NC_CACHE
    X = np.asarray(neighbour_states, np.float32)
    wpb = _host_pack_weights(G, U_stack, U_output)

    in_maps = []
    for c in range(NCORES):
        xmc, xtc = _pack_x_core(X[c * NPC:(c + 1) * NPC])
        in_maps.append({"xm": xmc, "xt": xtc, "wp": wpb})

    if _NC_CACHE is None:
        _NC_CACHE = _build_nc()
    nc = _NC_CACHE

    trace = bool(os.environ.get("HOSVD_TRACE"))
    if trace:
        _install_ntff_hook()
    res = run_bass_kernel_spmd(nc, in_maps, core_ids=list(range(NCORES)),
                               trace=trace)
    if trace and res.exec_time_ns is not None:
        print(f"HW exec time: {res.exec_time_ns} ns")

    out = np.empty((N, 128), np.float32)
    for c in range(NCORES):
        ymc = np.asarray(res.results[c]["ym"], dtype=np.float32)  # [12,128,512]
        ytc = np.asarray(res.results[c]["yt"], dtype=np.float32)  # [128,106]
        base = c * NPC
        out[base:base + NSUP * T] = ymc.transpose(0, 2, 1).reshape(NSUP * T, 128)
        out[base + NSUP * T:base + NPC] = ytc.T
    return out
